# revision 27
# baseline (speedup 1.0000x reference)
"""Trainium2 Bass kernel for AdaptiveEmbedding T2I sims.

Reference computation (per full batch):
  cap_repr = ragged-mean(cap_embed, lens)                       (Bc, D)
  bn       = batchnorm(img_embed^T) over (Bi, R) per channel d  (Bi, D, R)
  gamma    = MLP_g(cap_repr); beta = MLP_b(cap_repr)            (Bc, D)
  out      = bn * gamma + beta                                  (Bc, Bi, D, R)
  m        = softmax(out * 10, axis=-1)
  img_vec  = l2norm(mean_r(m * out))                            (Bc, Bi, D)
  sims     = einsum('cbd,cd->bc', img_vec, l2norm(cap_repr))    (Bi, Bc)

Device algebra (what the kernel actually computes, per caption c):
  softmax weights are invariant to the +beta shift and to any per-(c,d)
  constant factor, so with A = G*gamma*rstd:
     e = exp(A * x)          x = imgT[d, (b,r)]   (raw image, d on partitions)
     S' = sum_r(e*x) / sum_r(e)
     iv = P1*S' + P2         P1 = gamma*rstd, P2 = gamma*cbn + beta
  iv = R * img_vec(un-normalized);  sims = s1 / ((sqrt(s2)+R*eps)(sqrt(s3)+eps))
  with s1 = sum_d iv*cv, s2 = sum_d iv^2, s3 = sum_d cv^2.

Sharding: captions (Bc=32) split 4-per-core across 8 cores; img + MLP params
replicated. Per-core output is its 4 sims columns; host concatenates.

Engine assignment (v2): ScalarE exp+stats-accum passes; DVE muls + segmented
r-sum trees (the bottleneck — everything else is kept off its queue);
GpSimd the tiny scalar stat chain + lhs8 init; PE the cap/MLP matmuls and
the s1/s2/s3 PSUM reductions.
"""

import sys

if "/opt/trn_rl_repo" not in sys.path:
    sys.path.insert(0, "/opt/trn_rl_repo")

import numpy as np

# Problem constants (hardcoded per spec)
Bi, R, D, Bc, T, H = 64, 36, 1024, 32, 64, 128
NCORES = 8
CL = Bc // NCORES            # captions per core = 4
BR = Bi * R                  # 2304
P = 128                      # partitions
ND = D // P                  # 8 d-chunks
GAMMA = 10.0
EPS_BN = 1e-5
EPS_L2 = 1e-8

_COMPILED = None             # cached (nc,) so repeat kernel() calls skip rebuild


def _patch_act_tables():
    """Steer the act-table chooser to `natural_log_exp_and_others` (the only
    set with both exp and ln) for every function this kernel uses, so the
    Scalar engine never swaps table sets mid-kernel (~2.7us per swap)."""
    from concourse import bacc, hw_specs, mybir

    if getattr(bacc, "_act_tables_patched", False):
        return
    orig = hw_specs.get_activation_tables
    AF = mybir.ActivationFunctionType
    mine = {AF.Exp, AF.Ln, AF.Copy, AF.Square, AF.Identity, AF.Relu}

    def patched(arch):
        tables = orig(arch)
        for name, funcs in tables.items():
            if name != "natural_log_exp_and_others":
                tables[name] = funcs - mine
        return tables

    bacc.get_activation_tables = patched
    bacc._act_tables_patched = True


def _build_graph():
    from concourse import bacc, mybir, tile
    import concourse.bass as bass

    _patch_act_tables()

    F32 = mybir.dt.float32
    BF16 = mybir.dt.bfloat16
    AF = mybir.ActivationFunctionType
    AX = mybir.AxisListType
    ALU = mybir.AluOpType

    nc = bacc.Bacc("TRN2", target_bir_lowering=False, debug=False,
                   num_devices=NCORES)

    imgT = nc.declare_dram_parameter("imgT", [D, BR], F32, isOutput=False)
    cap = nc.declare_dram_parameter("cap", [CL * T, D], F32, isOutput=False)
    wm = nc.declare_dram_parameter("wm", [CL * T, CL], F32, isOutput=False)
    Wg1 = nc.declare_dram_parameter("Wg1", [D, H], F32, isOutput=False)
    Wg2 = nc.declare_dram_parameter("Wg2", [H, D], F32, isOutput=False)
    Wb1 = nc.declare_dram_parameter("Wb1", [D, H], F32, isOutput=False)
    Wb2 = nc.declare_dram_parameter("Wb2", [H, D], F32, isOutput=False)
    bias_pack = nc.declare_dram_parameter("bias_pack", [P, 2 + 2 * ND], F32,
                                          isOutput=False)
    lhs8s = nc.declare_dram_parameter("lhs8s", [P, ND * CL * 8], F32,
                                      isOutput=False)
    out_ext = nc.declare_dram_parameter("out", [CL, Bi], F32, isOutput=True)

    with tile.TileContext(nc) as tc:
        with (
            tc.tile_pool(name="xfpool", bufs=3) as xfp,
            tc.tile_pool(name="xbpool", bufs=3) as xbp,
            tc.tile_pool(name="smallpool", bufs=1) as smallp,
            tc.tile_pool(name="epool", bufs=2) as ep,
            tc.tile_pool(name="wspool", bufs=1) as wsp,
            tc.tile_pool(name="vpool", bufs=2) as vp,
            tc.tile_pool(name="junkpool", bufs=2) as jp,
            tc.tile_pool(name="psum", bufs=3, space=bass.MemorySpace.PSUM) as pp,
            tc.tile_pool(name="psum_acc", bufs=1, space=bass.MemorySpace.PSUM) as ppa,
            tc.tile_pool(name="psum_s", bufs=1, space=bass.MemorySpace.PSUM) as pps,
        ):
            # ---------- loads. DMA issue instructions cost ~630ns EACH on
            # the issuing engine's queue, so the head-critical wave is
            # round-robined across the three HW-DGE queues (SP, ScalarE,
            # DVE); everything later trails on SP alone. ----------
            x_t = [None] * ND
            rr = [0]
            dma_engs = [nc.sync, nc.scalar]

            def dma(dst, src, wave):
                if wave == 1:
                    eng = dma_engs[rr[0] % 2]
                    rr[0] += 1
                else:
                    eng = nc.sync
                eng.dma_start(dst, src)

            def emit_img_dma(dc, nsplit=4, wave=3):
                xt = xfp.tile([P, BR], F32, tag="xall")
                x_t[dc] = xt
                w = BR // nsplit
                for k in range(nsplit):
                    dma(xt[:, k * w:(k + 1) * w],
                        imgT[dc * P:(dc + 1) * P, k * w:(k + 1) * w], wave)

            # wave 1: img chunk 0 (gates stats 0) and wm + cap (gate crT)
            # first, then gamma-MLP weights. Caption halves ride in one
            # issue each via a rearranged DRAM view; biases ship packed.
            cap_sb = smallp.tile([P, 2, D], F32)
            wm_sb = smallp.tile([P, 2, CL], F32)
            xt0 = xfp.tile([P, BR], F32, tag="xall")
            x_t[0] = xt0
            capv = cap[:, :].rearrange("(ct p) c -> p ct c", p=P)
            for k in range(8):
                dma(xt0[:, k * 288:(k + 1) * 288],
                    imgT[0:P, k * 288:(k + 1) * 288], 1)
                if k < 2:
                    dma(wm_sb[:, k, :], wm[k * P:(k + 1) * P, :], 1)
                dma(cap_sb[:, :, k * 128:(k + 1) * 128],
                    capv[:, :, k * 128:(k + 1) * 128], 1)
            wg1_sb = smallp.tile([P, ND, H], F32)
            wb1_sb = smallp.tile([P, ND, H], F32)
            wg1v = Wg1[:, :].rearrange("(a p) h -> p a h", p=P)
            wb1v = Wb1[:, :].rearrange("(a p) h -> p a h", p=P)
            for k in range(4):
                dma(wg1_sb[:, 2 * k:2 * k + 2, :], wg1v[:, 2 * k:2 * k + 2, :],
                    1)
            wg2_sb = smallp.tile([P, D], F32)
            wb2_sb = smallp.tile([P, D], F32)
            for k in range(4):
                dma(wg2_sb[:, k * 256:(k + 1) * 256],
                    Wg2[:, k * 256:(k + 1) * 256], 1)
            biases_sb = smallp.tile([P, 2 + 2 * ND], F32)
            dma(biases_sb[:], bias_pack[:, :], 1)
            bg1_sb = biases_sb[:, 0:1]
            bb1_sb = biases_sb[:, 1:2]
            bg2t_sb = biases_sb[:, 2:2 + ND]
            bb2t_sb = biases_sb[:, 2 + ND:2 + 2 * ND]

            emit_img_dma(1, nsplit=6, wave=1)
            # wave 2 (SP queue): later img chunks, beta-MLP weights, lhs8s
            emit_img_dma(2, nsplit=6, wave=2)
            for k in range(4):
                dma(wb1_sb[:, 2 * k:2 * k + 2, :], wb1v[:, 2 * k:2 * k + 2, :],
                    2)
            for k in range(4):
                dma(wb2_sb[:, k * 256:(k + 1) * 256],
                    Wb2[:, k * 256:(k + 1) * 256], 2)

            eps_bn_sb = smallp.tile([P, 1], F32)
            nc.gpsimd.memset(eps_bn_sb[:], BR * EPS_BN)
            expb_sb = smallp.tile([P, 1], F32)
            nc.gpsimd.memset(expb_sb[:], 0.5 * float(np.log(BR)) +
                             float(np.log(GAMMA)))
            eps_l2_sb = smallp.tile([P, 1], F32)
            nc.gpsimd.memset(eps_l2_sb[:], EPS_L2)
            eps_rl2_sb = smallp.tile([P, 1], F32)
            nc.gpsimd.memset(eps_rl2_sb[:], R * GAMMA * EPS_L2)

            # ---------- BN stats tiles (filled per-dchunk inside the main
            # loop so stats for chunk k+1 overlap compute on chunk k) ----------
            sumx = smallp.tile([P, ND], F32)
            sumsq = smallp.tile([P, ND], F32)
            lnv = smallp.tile([P, ND], F32)
            cbn = smallp.tile([P, ND], F32)
            grstd = smallp.tile([P, ND], F32)

            # ---------- cap_repr^T [d, c] directly (pipelines with the cap
            # DMA: crT[dc] is ready two matmuls after its columns land) ------
            crT = smallp.tile([P, ND, CL], F32)
            for dc in range(ND):
                pcr = pp.tile([P, CL], F32, tag="pcr")
                for ct in range(2):
                    nc.tensor.matmul(pcr[:], cap_sb[:, ct, dc * P:(dc + 1) * P],
                                     wm_sb[:, ct, :],
                                     start=(ct == 0), stop=(ct == 1))
                nc.vector.tensor_copy(crT[:, dc, :], pcr[:])

            # ---------- conditioning MLPs, all in transposed form ----------
            # hT[H, c] accumulates lhsT=W1-chunk x rhs=crT-chunk over dc;
            # gammaT[d-chunk, c] = lhsT=W2[:, d-chunk] x rhs=hT. No PE
            # transposes or [c, D] intermediates needed.
            gammaT = smallp.tile([P, ND, CL], F32)
            betaT = smallp.tile([P, ND, CL], F32)
            for (w1s, w2s, b1s, b2s, dstT, tg) in (
                (wg1_sb, wg2_sb, bg1_sb, bg2t_sb, gammaT, "g"),
                (wb1_sb, wb2_sb, bb1_sb, bb2t_sb, betaT, "b"),
            ):
                phT = ppa.tile([H, CL], F32, tag="ph" + tg)
                for dc in range(ND):
                    nc.tensor.matmul(phT[:], w1s[:, dc, :], crT[:, dc, :],
                                     start=(dc == 0), stop=(dc == ND - 1))
                hT = smallp.tile([H, CL], F32, tag="hT" + tg)
                nc.vector.tensor_scalar(hT[:], phT[:], b1s, 0.0,
                                        op0=ALU.add, op1=ALU.max)
                for dc in range(ND):
                    pg = pp.tile([P, CL], F32, tag="pcr")
                    nc.tensor.matmul(pg[:], w2s[:, dc * P:(dc + 1) * P],
                                     hT[:], start=True, stop=True)
                    nc.vector.tensor_scalar(dstT[:, dc, :], pg[:],
                                            b2s[:, dc:dc + 1], None,
                                            op0=ALU.add)

            # ---------- A, P2 tiles (filled per-dchunk in main loop) ----
            A = smallp.tile([P, ND, CL], F32)
            P2 = smallp.tile([P, ND, CL], F32)

            # ---------- masked-column lhsT tiles ----------
            # lhs8[:, dc, c, :]: col c holds 1 (others 0), col 4+c holds
            # cap_repr column c. One matmul against rhs=[iv | iv2] then
            # accumulates s1 into PSUM row c and s2 into row 4+c with zero
            # contributions elsewhere. The static one-hot part ships from
            # DRAM; the 4 cap_repr diagonals are strided ScalarE copies.
            lhs8 = smallp.tile([P, ND, CL, 8], F32)
            nc.sync.dma_start(lhs8[:].rearrange("p a b c -> p (a b c)"),
                              lhs8s[:, :])
            for c in range(CL):
                nc.vector.tensor_copy(lhs8[:, :, c, 4 + c:5 + c],
                                      crT[:, :, c:c + 1])

            # ---------- main loop ----------
            G4 = 8 * Bi   # e/q slabs of all 4 captions in one shared tree

            def tree_reduce(dst, src):
                """dst[P, 8*Bi] (fp32) = segmented sum over r of src[P, 8, Bi*R]
                (bf16, slabs e_c0|q_c0|..|e_c3|q_c3) via a binary tree of
                2x-mode tensor_tensor adds."""
                s4 = src[:].rearrange("p a (b r) -> p a b r", r=R)
                t16 = wsp.tile([P, G4, 16], BF16, tag="t16")
                nc.vector.tensor_add(t16[:], s4[:, :, :, 0:16], s4[:, :, :, 16:32])
                t8 = wsp.tile([P, G4, 8], BF16, tag="t8")
                nc.vector.tensor_add(t8[:], t16[:, :, 0:8], t16[:, :, 8:16])
                t4 = wsp.tile([P, G4, 4], BF16, tag="t4")
                nc.vector.tensor_add(t4[:], t8[:, :, 0:4], t8[:, :, 4:8])
                t4b = wsp.tile([P, G4, 4], BF16, tag="t4b")
                nc.vector.tensor_add(
                    t4b[:], t4[:],
                    s4[:, :, :, 32:36].rearrange("p a b r -> p (a b) r"))
                t2 = wsp.tile([P, G4, 2], BF16, tag="t2")
                nc.vector.tensor_add(t2[:], t4b[:, :, 0:2], t4b[:, :, 2:4])
                nc.vector.tensor_add(
                    dst[:].rearrange("p (g o) -> p g o", o=1),
                    t2[:, :, 0:1], t2[:, :, 1:2])

            sims_sb = smallp.tile([CL, Bi], F32)
            ps12 = pps.tile([2 * CL, 2 * Bi], F32, tag="s12")
            ps3 = ppa.tile([CL, 1], F32, tag="s3")

            def emit_stats(dc):
                """Per-dchunk BN stats + affine coefficients. Emitted one
                group ahead of its consumers. The two accumulate passes run
                on ScalarE (the copy also produces the bf16 x used by the
                DVE muls); the scalar chain runs on the otherwise-idle
                GpSimd engine."""
                d1 = dc + 1
                xbf = xbp.tile([P, BR], BF16, tag="xbf")
                x_bf_t[dc] = xbf
                nc.scalar.activation(xbf[:], x_t[dc][:], AF.Copy,
                                     accum_out=sumx[:, dc:d1])
                junk = jp.tile([P, BR], BF16, tag="junk")
                nc.scalar.activation(junk[:], x_t[dc][:], AF.Square,
                                     accum_out=sumsq[:, dc:d1])
                # BR*var = sumsq - sumx*mean in two tiny DVE ops; the 1/BR
                # and GAMMA factors fold into the Exp bias (grstd = G*rstd =
                # exp(-0.5*ln(BR*var + BR*eps) + 0.5*ln(BR) + ln(G))). The
                # GAMMA on iv is harmless: sims is invariant to uniform iv
                # scaling, and the host pre-scales Wb2/bb2 by GAMMA to match.
                nmean = wsp.tile([P, 1], F32, tag="nmean")
                nc.vector.tensor_scalar_mul(nmean[:], sumx[:, dc:d1],
                                            -1.0 / BR)
                nc.vector.scalar_tensor_tensor(lnv[:, dc:d1], sumx[:, dc:d1],
                                               nmean[:], sumsq[:, dc:d1],
                                               ALU.mult, ALU.add)
                nc.scalar.activation(lnv[:, dc:d1], lnv[:, dc:d1], AF.Ln,
                                     bias=eps_bn_sb[:])
                nc.scalar.activation(grstd[:, dc:d1], lnv[:, dc:d1], AF.Exp,
                                     scale=-0.5, bias=expb_sb[:])
                nc.vector.tensor_mul(cbn[:, dc:d1], nmean[:],
                                     grstd[:, dc:d1])
                nc.scalar.activation(A[:, dc, :], gammaT[:, dc, :], AF.Copy,
                                     scale=grstd[:, dc:d1])

            def emit_p2(dc):
                """P2 = gammaT*cbn + betaT. Emitted AFTER the dc's exps so
                the ScalarE queue never blocks on the (late-arriving)
                beta-MLP weights."""
                nc.scalar.activation(P2[:, dc, :], gammaT[:, dc, :], AF.Copy,
                                     scale=cbn[:, dc:dc + 1])
                nc.vector.tensor_add(P2[:, dc, :], P2[:, dc, :],
                                     betaT[:, dc, :])

            # ---------- s3[c] = sum_d cv^2 ----------
            # emitted mid-main-loop (group 2): late enough that the scheduler
            # cannot hoist these 32 PE matmuls ahead of the MLP chain, early
            # enough that ps3 is long done when the epilogue needs it
            def emit_s3():
                for c in range(CL):
                    for dcc in range(ND):
                        nc.tensor.matmul(ps3[:], lhs8[:, dcc, c, CL:2 * CL],
                                         crT[:, dcc, c:c + 1],
                                         start=(c == 0 and dcc == 0),
                                         stop=(c == CL - 1 and dcc == ND - 1))

            x_bf_t = [None] * ND
            emit_stats(0)
            for dc in range(ND):
                if dc + 3 < ND:
                    emit_img_dma(dc + 3)
                if dc == 2:
                    emit_s3()
                eq = ep.tile([P, 2 * CL, BR], BF16, tag="eq")
                for c in range(CL):
                    # next group's stats after the first two exps: one group
                    # of lookahead without delaying the current chunk's work
                    if c == 2 and dc + 1 < ND:
                        emit_stats(dc + 1)
                    nc.scalar.activation(eq[:, 2 * c, :], x_t[dc][:],
                                         AF.Exp,
                                         scale=A[:, dc, c:c + 1])
                emit_p2(dc)
                # q slabs for all 4 captions in one broadcast mul (slabs
                # 1,3,5,7 = e-slabs 0,2,4,6 times x)
                eqv = eq[:].rearrange("p (a s) n -> p a s n", s=2)
                xb = x_bf_t[dc][:].unsqueeze(1)
                nc.vector.tensor_mul(eqv[:, :, 1, :], eqv[:, :, 0, :],
                                     xb.broadcast_to([P, CL, BR]))
                sesq = wsp.tile([P, G4], F32, tag="sesq")
                tree_reduce(sesq, eq)
                s4v = sesq[:].rearrange("p (a e b) -> p a e b", a=CL, e=2)
                rec = wsp.tile([P, CL, Bi], F32, tag="rec")
                nc.vector.reciprocal_approx_fast(rec[:], s4v[:, :, 0, :])
                Sp = wsp.tile([P, CL, Bi], F32, tag="Sp")
                nc.vector.tensor_mul(Sp[:], s4v[:, :, 1, :], rec[:])
                for c in range(CL):
                    iviv2 = vp.tile([P, 2, Bi], F32, tag="iviv2")
                    nc.vector.tensor_scalar(iviv2[:, 0, :], Sp[:, c, :],
                                            A[:, dc, c:c + 1],
                                            P2[:, dc, c:c + 1],
                                            op0=ALU.mult, op1=ALU.add)
                    nc.scalar.activation(iviv2[:, 1, :], iviv2[:, 0, :],
                                         AF.Square)
                    nc.tensor.matmul(
                        ps12[:], lhs8[:, dc, c, :],
                        iviv2[:].rearrange("p a b -> p (a b)"),
                        start=(dc == 0 and c == 0),
                        stop=(dc == ND - 1 and c == CL - 1))
            s1s2_sb = smallp.tile([2 * CL, 2 * Bi], F32)
            nc.vector.tensor_copy(s1s2_sb[:], ps12[:])
            s1_sb = smallp.tile([CL, Bi], F32)
            nc.sync.dma_start(s1_sb[:], s1s2_sb[CL:2 * CL, 0:Bi])
            # sq3 = sqrt(s3)+eps (Ln/Exp stay in the one loaded table set;
            # the ps3 dependency pins these after the main loop)
            sq3 = smallp.tile([CL, 1], F32)
            nc.scalar.activation(sq3[:], ps3[:], AF.Ln)
            nc.scalar.activation(sq3[:], sq3[:], AF.Exp, scale=0.5)
            nc.scalar.add(sq3[:], sq3[:], eps_l2_sb[:CL])
            den = wsp.tile([CL, Bi], F32, tag="den")
            nc.scalar.activation(den[:], s1s2_sb[0:CL, Bi:2 * Bi], AF.Ln)
            nc.scalar.activation(den[:], den[:], AF.Exp, scale=0.5)
            nc.scalar.add(den[:], den[:], eps_rl2_sb[:CL])
            nc.vector.tensor_scalar_mul(den[:], den[:], sq3[:])
            rden = wsp.tile([CL, Bi], F32, tag="rden")
            nc.vector.reciprocal_approx_fast(rden[:], den[:])
            nc.vector.tensor_mul(sims_sb[:], rden[:], s1_sb[:])

            nc.sync.dma_start(out_ext[:, :], sims_sb[:])

    nc.compile()
    return nc


def _prep_inputs(img_embed, cap_embed, Wg1, bg1, Wg2, bg2, Wb1, bb1, Wb2, bb2,
                 lens):
    """Host-side layout prep + per-core sharding. Returns in_maps (list of 8)."""
    f32 = np.float32
    imgT = np.ascontiguousarray(
        np.transpose(np.asarray(img_embed, f32), (2, 0, 1))).reshape(D, BR)
    capf = np.asarray(cap_embed, f32)
    lensf = np.asarray(lens)
    # ragged-mean weights: wfull[c, t] = (t < lens[c]) / lens[c]
    wfull = ((np.arange(T)[None, :] < lensf[:, None]) /
             lensf[:, None].astype(f32)).astype(f32)

    # one-hot static part of the masked-column lhsT tiles
    lhs8s = np.zeros((P, ND, CL, 8), f32)
    for c in range(CL):
        lhs8s[:, :, c, c] = 1.0
    shared = {
        "imgT": imgT,
        "Wg1": np.ascontiguousarray(np.asarray(Wg1, f32)),
        "Wg2": np.ascontiguousarray(np.asarray(Wg2, f32)),
        "Wb1": np.ascontiguousarray(np.asarray(Wb1, f32)),
        # beta path pre-scaled by GAMMA: the kernel computes iv' = GAMMA*iv,
        # to which sims is invariant (uniform scale cancels in s1/sqrt(s2))
        "Wb2": np.ascontiguousarray(np.asarray(Wb2, f32) * GAMMA),
        "bias_pack": np.ascontiguousarray(np.concatenate([
            np.asarray(bg1, f32).reshape(H, 1),
            np.asarray(bb1, f32).reshape(H, 1),
            np.asarray(bg2, f32).reshape(ND, P).T,
            np.asarray(bb2, f32).reshape(ND, P).T * GAMMA,
        ], axis=1)),
        "lhs8s": np.ascontiguousarray(lhs8s.reshape(P, ND * CL * 8)),
    }
    in_maps = []
    for i in range(NCORES):
        cs = slice(i * CL, (i + 1) * CL)
        cap_local = np.ascontiguousarray(capf[cs].reshape(CL * T, D))
        # block-diagonal mask-weight matrix [(c,t), c']
        wmat = np.zeros((CL * T, CL), f32)
        for cl in range(CL):
            wmat[cl * T:(cl + 1) * T, cl] = wfull[i * CL + cl]
        in_maps.append({**shared, "cap": cap_local, "wm": wmat})
    return in_maps


def kernel(**inputs) -> np.ndarray:
    global _COMPILED
    from concourse.bass_utils import run_bass_kernel_spmd

    if _COMPILED is None:
        _COMPILED = _build_graph()
    nc = _COMPILED

    in_maps = _prep_inputs(**inputs)
    res = run_bass_kernel_spmd(nc, in_maps, core_ids=list(range(NCORES)))
    sims = np.empty((Bi, Bc), np.float32)
    for i in range(NCORES):
        sims[:, i * CL:(i + 1) * CL] = res.results[i]["out"].T
    return sims


if __name__ == "__main__":
    # smoke test with random data
    rng = np.random.default_rng(0)
    ins = {
        "img_embed": rng.standard_normal((Bi, R, D), np.float32),
        "cap_embed": rng.standard_normal((Bc, T, D), np.float32),
        "Wg1": rng.standard_normal((D, H), np.float32) * 0.02,
        "bg1": np.zeros(H, np.float32),
        "Wg2": rng.standard_normal((H, D), np.float32) * 0.02,
        "bg2": np.zeros(D, np.float32),
        "Wb1": rng.standard_normal((D, H), np.float32) * 0.02,
        "bb1": np.zeros(H, np.float32),
        "Wb2": rng.standard_normal((H, D), np.float32) * 0.02,
        "bb2": np.zeros(D, np.float32),
        "lens": rng.integers(4, T - 4, Bc).astype(np.int32),
    }
    out = kernel(**ins)
    print(out.shape, out.dtype, np.abs(out).mean())


# revision 28
# speedup vs baseline: 1.0285x; 1.0285x over previous
"""Trainium2 Bass kernel for AdaptiveEmbedding T2I sims.

Reference computation (per full batch):
  cap_repr = ragged-mean(cap_embed, lens)                       (Bc, D)
  bn       = batchnorm(img_embed^T) over (Bi, R) per channel d  (Bi, D, R)
  gamma    = MLP_g(cap_repr); beta = MLP_b(cap_repr)            (Bc, D)
  out      = bn * gamma + beta                                  (Bc, Bi, D, R)
  m        = softmax(out * 10, axis=-1)
  img_vec  = l2norm(mean_r(m * out))                            (Bc, Bi, D)
  sims     = einsum('cbd,cd->bc', img_vec, l2norm(cap_repr))    (Bi, Bc)

Device algebra (what the kernel actually computes, per caption c):
  softmax weights are invariant to the +beta shift and to any per-(c,d)
  constant factor, so with A = G*gamma*rstd:
     e = exp(A * x)          x = imgT[d, (b,r)]   (raw image, d on partitions)
     S' = sum_r(e*x) / sum_r(e)
     iv = P1*S' + P2         P1 = gamma*rstd, P2 = gamma*cbn + beta
  iv = R * img_vec(un-normalized);  sims = s1 / ((sqrt(s2)+R*eps)(sqrt(s3)+eps))
  with s1 = sum_d iv*cv, s2 = sum_d iv^2, s3 = sum_d cv^2.

Sharding: captions (Bc=32) split 4-per-core across 8 cores; img + MLP params
replicated. Per-core output is its 4 sims columns; host concatenates.

Engine assignment (v2): ScalarE exp+stats-accum passes; DVE muls + segmented
r-sum trees (the bottleneck — everything else is kept off its queue);
GpSimd the tiny scalar stat chain + lhs8 init; PE the cap/MLP matmuls and
the s1/s2/s3 PSUM reductions.
"""

import sys

if "/opt/trn_rl_repo" not in sys.path:
    sys.path.insert(0, "/opt/trn_rl_repo")

import numpy as np

# Problem constants (hardcoded per spec)
Bi, R, D, Bc, T, H = 64, 36, 1024, 32, 64, 128
NCORES = 8
CL = Bc // NCORES            # captions per core = 4
BR = Bi * R                  # 2304
P = 128                      # partitions
ND = D // P                  # 8 d-chunks
GAMMA = 10.0
EPS_BN = 1e-5
EPS_L2 = 1e-8

_COMPILED = None             # cached (nc,) so repeat kernel() calls skip rebuild


def _patch_act_tables():
    """Steer the act-table chooser to `natural_log_exp_and_others` (the only
    set with both exp and ln) for every function this kernel uses, so the
    Scalar engine never swaps table sets mid-kernel (~2.7us per swap)."""
    from concourse import bacc, hw_specs, mybir

    if getattr(bacc, "_act_tables_patched", False):
        return
    orig = hw_specs.get_activation_tables
    AF = mybir.ActivationFunctionType
    mine = {AF.Exp, AF.Ln, AF.Copy, AF.Square, AF.Identity, AF.Relu}

    def patched(arch):
        tables = orig(arch)
        for name, funcs in tables.items():
            if name != "natural_log_exp_and_others":
                tables[name] = funcs - mine
        return tables

    bacc.get_activation_tables = patched
    bacc._act_tables_patched = True


def _build_graph():
    from concourse import bacc, mybir, tile
    import concourse.bass as bass

    _patch_act_tables()

    F32 = mybir.dt.float32
    BF16 = mybir.dt.bfloat16
    AF = mybir.ActivationFunctionType
    AX = mybir.AxisListType
    ALU = mybir.AluOpType

    nc = bacc.Bacc("TRN2", target_bir_lowering=False, debug=False,
                   num_devices=NCORES)

    imgT = nc.declare_dram_parameter("imgT", [D, BR], F32, isOutput=False)
    cap = nc.declare_dram_parameter("cap", [CL * T, D], F32, isOutput=False)
    wm = nc.declare_dram_parameter("wm", [CL * T, CL], F32, isOutput=False)
    Wg1 = nc.declare_dram_parameter("Wg1", [D, H], F32, isOutput=False)
    Wg2 = nc.declare_dram_parameter("Wg2", [H, D], F32, isOutput=False)
    Wb1 = nc.declare_dram_parameter("Wb1", [D, H], F32, isOutput=False)
    Wb2 = nc.declare_dram_parameter("Wb2", [H, D], F32, isOutput=False)
    bias_pack = nc.declare_dram_parameter("bias_pack", [P, 2 + 2 * ND], F32,
                                          isOutput=False)
    lhs8s = nc.declare_dram_parameter("lhs8s", [P, ND * CL * 8], F32,
                                      isOutput=False)
    out_ext = nc.declare_dram_parameter("out", [CL, Bi], F32, isOutput=True)

    with tile.TileContext(nc) as tc:
        with (
            tc.tile_pool(name="xfpool", bufs=3) as xfp,
            tc.tile_pool(name="xbpool", bufs=3) as xbp,
            tc.tile_pool(name="smallpool", bufs=1) as smallp,
            tc.tile_pool(name="epool", bufs=2) as ep,
            tc.tile_pool(name="wspool", bufs=1) as wsp,
            tc.tile_pool(name="vpool", bufs=2) as vp,
            tc.tile_pool(name="junkpool", bufs=2) as jp,
            tc.tile_pool(name="psum", bufs=3, space=bass.MemorySpace.PSUM) as pp,
            tc.tile_pool(name="psum_acc", bufs=1, space=bass.MemorySpace.PSUM) as ppa,
            tc.tile_pool(name="psum_s", bufs=1, space=bass.MemorySpace.PSUM) as pps,
        ):
            # ---------- loads. DMA issue instructions cost ~630ns EACH on
            # the issuing engine's queue, so the head-critical wave is
            # round-robined across the three HW-DGE queues (SP, ScalarE,
            # DVE); everything later trails on SP alone. ----------
            x_t = [None] * ND
            rr = [0]
            dma_engs = [nc.sync, nc.scalar]

            def dma(dst, src, wave):
                if wave == 1:
                    eng = dma_engs[rr[0] % 2]
                    rr[0] += 1
                else:
                    eng = nc.sync
                eng.dma_start(dst, src)

            def emit_img_dma(dc, nsplit=4, wave=3):
                xt = xfp.tile([P, BR], F32, tag="xall")
                x_t[dc] = xt
                w = BR // nsplit
                for k in range(nsplit):
                    dma(xt[:, k * w:(k + 1) * w],
                        imgT[dc * P:(dc + 1) * P, k * w:(k + 1) * w], wave)

            # wave 1: img chunk 0 (gates stats 0) and wm + cap (gate crT)
            # first, then gamma-MLP weights. Caption halves ride in one
            # issue each via a rearranged DRAM view; biases ship packed.
            cap_sb = smallp.tile([P, 2, D], F32)
            wm_sb = smallp.tile([P, 2, CL], F32)
            xt0 = xfp.tile([P, BR], F32, tag="xall")
            x_t[0] = xt0
            capv = cap[:, :].rearrange("(ct p) c -> p ct c", p=P)
            for k in range(8):
                dma(xt0[:, k * 288:(k + 1) * 288],
                    imgT[0:P, k * 288:(k + 1) * 288], 1)
                if k < 2:
                    dma(wm_sb[:, k, :], wm[k * P:(k + 1) * P, :], 1)
                dma(cap_sb[:, :, k * 128:(k + 1) * 128],
                    capv[:, :, k * 128:(k + 1) * 128], 1)
            wg1_sb = smallp.tile([P, ND, H], F32)
            wb1_sb = smallp.tile([P, ND, H], F32)
            wg1v = Wg1[:, :].rearrange("(a p) h -> p a h", p=P)
            wb1v = Wb1[:, :].rearrange("(a p) h -> p a h", p=P)
            for k in range(4):
                dma(wg1_sb[:, 2 * k:2 * k + 2, :], wg1v[:, 2 * k:2 * k + 2, :],
                    1)
            wg2_sb = smallp.tile([P, D], F32)
            wb2_sb = smallp.tile([P, D], F32)
            for k in range(4):
                dma(wg2_sb[:, k * 256:(k + 1) * 256],
                    Wg2[:, k * 256:(k + 1) * 256], 1)
            biases_sb = smallp.tile([P, 2 + 2 * ND], F32)
            dma(biases_sb[:], bias_pack[:, :], 1)
            bg1_sb = biases_sb[:, 0:1]
            bb1_sb = biases_sb[:, 1:2]
            bg2t_sb = biases_sb[:, 2:2 + ND]
            bb2t_sb = biases_sb[:, 2 + ND:2 + 2 * ND]

            emit_img_dma(1, nsplit=6, wave=1)
            # wave 2 (SP queue): later img chunks, beta-MLP weights, lhs8s
            emit_img_dma(2, nsplit=6, wave=2)
            for k in range(4):
                dma(wb1_sb[:, 2 * k:2 * k + 2, :], wb1v[:, 2 * k:2 * k + 2, :],
                    2)
            for k in range(4):
                dma(wb2_sb[:, k * 256:(k + 1) * 256],
                    Wb2[:, k * 256:(k + 1) * 256], 2)

            eps_bn_sb = smallp.tile([P, 1], F32)
            nc.gpsimd.memset(eps_bn_sb[:], BR * EPS_BN)
            expb_sb = smallp.tile([P, 1], F32)
            nc.gpsimd.memset(expb_sb[:], 0.5 * float(np.log(BR)) +
                             float(np.log(GAMMA)))
            eps_l2_sb = smallp.tile([P, 1], F32)
            nc.gpsimd.memset(eps_l2_sb[:], EPS_L2)
            eps_rl2_sb = smallp.tile([P, 1], F32)
            nc.gpsimd.memset(eps_rl2_sb[:], R * GAMMA * EPS_L2)

            # ---------- BN stats tiles (filled per-dchunk inside the main
            # loop so stats for chunk k+1 overlap compute on chunk k) ----------
            sumx = smallp.tile([P, ND], F32)
            sumsq = smallp.tile([P, ND], F32)
            lnv = smallp.tile([P, ND], F32)
            cbn = smallp.tile([P, ND], F32)
            grstd = smallp.tile([P, ND], F32)

            # ---------- cap_repr^T [d, c] directly (pipelines with the cap
            # DMA: crT[dc] is ready two matmuls after its columns land) ------
            crT = smallp.tile([P, ND, CL], F32)
            for dc in range(ND):
                pcr = pp.tile([P, CL], F32, tag="pcr")
                for ct in range(2):
                    nc.tensor.matmul(pcr[:], cap_sb[:, ct, dc * P:(dc + 1) * P],
                                     wm_sb[:, ct, :],
                                     start=(ct == 0), stop=(ct == 1))
                nc.vector.tensor_copy(crT[:, dc, :], pcr[:])

            # ---------- conditioning MLPs, all in transposed form ----------
            # hT[H, c] accumulates lhsT=W1-chunk x rhs=crT-chunk over dc;
            # gammaT[d-chunk, c] = lhsT=W2[:, d-chunk] x rhs=hT. No PE
            # transposes or [c, D] intermediates needed.
            gammaT = smallp.tile([P, ND, CL], F32)
            betaT = smallp.tile([P, ND, CL], F32)
            for (w1s, w2s, b1s, b2s, dstT, tg) in (
                (wg1_sb, wg2_sb, bg1_sb, bg2t_sb, gammaT, "g"),
                (wb1_sb, wb2_sb, bb1_sb, bb2t_sb, betaT, "b"),
            ):
                phT = ppa.tile([H, CL], F32, tag="ph" + tg)
                for dc in range(ND):
                    nc.tensor.matmul(phT[:], w1s[:, dc, :], crT[:, dc, :],
                                     start=(dc == 0), stop=(dc == ND - 1))
                hT = smallp.tile([H, CL], F32, tag="hT" + tg)
                nc.vector.tensor_scalar(hT[:], phT[:], b1s, 0.0,
                                        op0=ALU.add, op1=ALU.max)
                for dc in range(ND):
                    pg = pp.tile([P, CL], F32, tag="pcr")
                    nc.tensor.matmul(pg[:], w2s[:, dc * P:(dc + 1) * P],
                                     hT[:], start=True, stop=True)
                    nc.vector.tensor_scalar(dstT[:, dc, :], pg[:],
                                            b2s[:, dc:dc + 1], None,
                                            op0=ALU.add)

            # ---------- A, P2 tiles (filled per-dchunk in main loop) ----
            A = smallp.tile([P, ND, CL], F32)
            P2 = smallp.tile([P, ND, CL], F32)

            # ---------- masked-column lhsT tiles ----------
            # lhs8[:, dc, c, :]: col c holds 1 (others 0), col 4+c holds
            # cap_repr column c. One matmul against rhs=[iv | iv2] then
            # accumulates s1 into PSUM row c and s2 into row 4+c with zero
            # contributions elsewhere. The static one-hot part ships from
            # DRAM; the 4 cap_repr diagonals are strided ScalarE copies.
            lhs8 = smallp.tile([P, ND, CL, 8], F32)
            nc.sync.dma_start(lhs8[:].rearrange("p a b c -> p (a b c)"),
                              lhs8s[:, :])
            for c in range(CL):
                nc.vector.tensor_copy(lhs8[:, :, c, 4 + c:5 + c],
                                      crT[:, :, c:c + 1])

            # ---------- main loop ----------
            G4 = 8 * Bi   # e/q slabs of all 4 captions in one shared tree

            def tree_reduce(dst, src):
                """dst[P, 8*Bi] (fp32) = segmented sum over r of src[P, 8, Bi*R]
                (bf16, slabs e_c0|q_c0|..|e_c3|q_c3) via a binary tree of
                2x-mode tensor_tensor adds."""
                s4 = src[:].rearrange("p a (b r) -> p a b r", r=R)
                t16 = wsp.tile([P, G4, 16], BF16, tag="t16")
                nc.vector.tensor_add(t16[:], s4[:, :, :, 0:16], s4[:, :, :, 16:32])
                t8 = wsp.tile([P, G4, 8], BF16, tag="t8")
                nc.vector.tensor_add(t8[:], t16[:, :, 0:8], t16[:, :, 8:16])
                t4 = wsp.tile([P, G4, 4], BF16, tag="t4")
                nc.vector.tensor_add(t4[:], t8[:, :, 0:4], t8[:, :, 4:8])
                t4b = wsp.tile([P, G4, 4], BF16, tag="t4b")
                nc.vector.tensor_add(
                    t4b[:], t4[:],
                    s4[:, :, :, 32:36].rearrange("p a b r -> p (a b) r"))
                t2 = wsp.tile([P, G4, 2], BF16, tag="t2")
                nc.vector.tensor_add(t2[:], t4b[:, :, 0:2], t4b[:, :, 2:4])
                nc.vector.tensor_add(
                    dst[:].rearrange("p (g o) -> p g o", o=1),
                    t2[:, :, 0:1], t2[:, :, 1:2])

            sims_sb = smallp.tile([CL, Bi], F32)
            ps12 = pps.tile([2 * CL, 2 * Bi], F32, tag="s12")
            ps3 = ppa.tile([CL, 1], F32, tag="s3")

            def emit_stats(dc):
                """Per-dchunk BN stats + affine coefficients. Emitted one
                group ahead of its consumers. The two accumulate passes run
                on ScalarE (the copy also produces the bf16 x used by the
                DVE muls); the scalar chain runs on the otherwise-idle
                GpSimd engine."""
                d1 = dc + 1
                xbf = xbp.tile([P, BR], BF16, tag="xbf")
                x_bf_t[dc] = xbf
                nc.scalar.activation(xbf[:], x_t[dc][:], AF.Copy,
                                     accum_out=sumx[:, dc:d1])
                junk = jp.tile([P, BR], BF16, tag="junk")
                nc.scalar.activation(junk[:], x_t[dc][:], AF.Square,
                                     accum_out=sumsq[:, dc:d1])
                # BR*var = sumsq - sumx*mean in two tiny DVE ops; the 1/BR
                # and GAMMA factors fold into the Exp bias (grstd = G*rstd =
                # exp(-0.5*ln(BR*var + BR*eps) + 0.5*ln(BR) + ln(G))). The
                # GAMMA on iv is harmless: sims is invariant to uniform iv
                # scaling, and the host pre-scales Wb2/bb2 by GAMMA to match.
                nmean = wsp.tile([P, 1], F32, tag="nmean")
                nc.vector.tensor_scalar_mul(nmean[:], sumx[:, dc:d1],
                                            -1.0 / BR)
                nc.vector.scalar_tensor_tensor(lnv[:, dc:d1], sumx[:, dc:d1],
                                               nmean[:], sumsq[:, dc:d1],
                                               ALU.mult, ALU.add)
                nc.scalar.activation(lnv[:, dc:d1], lnv[:, dc:d1], AF.Ln,
                                     bias=eps_bn_sb[:])
                nc.scalar.activation(grstd[:, dc:d1], lnv[:, dc:d1], AF.Exp,
                                     scale=-0.5, bias=expb_sb[:])
                nc.vector.tensor_mul(cbn[:, dc:d1], nmean[:],
                                     grstd[:, dc:d1])
                nc.scalar.activation(A[:, dc, :], gammaT[:, dc, :], AF.Copy,
                                     scale=grstd[:, dc:d1])

            def emit_p2(dc):
                """P2 = gammaT*cbn + betaT. Emitted AFTER the dc's exps so
                the ScalarE queue never blocks on the (late-arriving)
                beta-MLP weights."""
                nc.scalar.activation(P2[:, dc, :], gammaT[:, dc, :], AF.Copy,
                                     scale=cbn[:, dc:dc + 1])
                nc.vector.tensor_add(P2[:, dc, :], P2[:, dc, :],
                                     betaT[:, dc, :])

            # ---------- s3[c] = sum_d cv^2 ----------
            # emitted mid-main-loop (group 2): late enough that the scheduler
            # cannot hoist these 32 PE matmuls ahead of the MLP chain, early
            # enough that ps3 is long done when the epilogue needs it
            def emit_s3():
                for c in range(CL):
                    for dcc in range(ND):
                        nc.tensor.matmul(ps3[:], lhs8[:, dcc, c, CL:2 * CL],
                                         crT[:, dcc, c:c + 1],
                                         start=(c == 0 and dcc == 0),
                                         stop=(c == CL - 1 and dcc == ND - 1))

            x_bf_t = [None] * ND
            emit_stats(0)
            for dc in range(ND):
                if dc + 3 < ND:
                    emit_img_dma(dc + 3)
                if dc == 2:
                    emit_s3()
                eq = ep.tile([P, 2 * CL, BR], BF16, tag="eq")
                eqv = eq[:].rearrange("p (a s) n -> p a s n", s=2)
                xb = x_bf_t[dc][:].unsqueeze(1)
                for c in range(CL):
                    # next group's stats after the first two exps: one group
                    # of lookahead without delaying the current chunk's work
                    if c == 2 and dc + 1 < ND:
                        emit_stats(dc + 1)
                    nc.scalar.activation(eq[:, 2 * c, :], x_t[dc][:],
                                         AF.Exp,
                                         scale=A[:, dc, c:c + 1])
                    if c % 2 == 1:
                        # q slabs for this caption pair in one broadcast mul
                        hh = c // 2
                        nc.vector.tensor_mul(
                            eqv[:, 2 * hh:2 * hh + 2, 1, :],
                            eqv[:, 2 * hh:2 * hh + 2, 0, :],
                            xb.broadcast_to([P, 2, BR]))
                emit_p2(dc)
                sesq = wsp.tile([P, G4], F32, tag="sesq")
                tree_reduce(sesq, eq)
                s4v = sesq[:].rearrange("p (a e b) -> p a e b", a=CL, e=2)
                rec = wsp.tile([P, CL, Bi], F32, tag="rec")
                nc.vector.reciprocal_approx_fast(rec[:], s4v[:, :, 0, :])
                Sp = wsp.tile([P, CL, Bi], F32, tag="Sp")
                nc.vector.tensor_mul(Sp[:], s4v[:, :, 1, :], rec[:])
                for c in range(CL):
                    iviv2 = vp.tile([P, 2, Bi], F32, tag="iviv2")
                    nc.vector.tensor_scalar(iviv2[:, 0, :], Sp[:, c, :],
                                            A[:, dc, c:c + 1],
                                            P2[:, dc, c:c + 1],
                                            op0=ALU.mult, op1=ALU.add)
                    nc.scalar.activation(iviv2[:, 1, :], iviv2[:, 0, :],
                                         AF.Square)
                    nc.tensor.matmul(
                        ps12[:], lhs8[:, dc, c, :],
                        iviv2[:].rearrange("p a b -> p (a b)"),
                        start=(dc == 0 and c == 0),
                        stop=(dc == ND - 1 and c == CL - 1))
            s1s2_sb = smallp.tile([2 * CL, 2 * Bi], F32)
            nc.vector.tensor_copy(s1s2_sb[:], ps12[:])
            s1_sb = smallp.tile([CL, Bi], F32)
            nc.sync.dma_start(s1_sb[:], s1s2_sb[CL:2 * CL, 0:Bi])
            # sq3 = sqrt(s3)+eps (Ln/Exp stay in the one loaded table set;
            # the ps3 dependency pins these after the main loop)
            sq3 = smallp.tile([CL, 1], F32)
            nc.scalar.activation(sq3[:], ps3[:], AF.Ln)
            nc.scalar.activation(sq3[:], sq3[:], AF.Exp, scale=0.5)
            nc.scalar.add(sq3[:], sq3[:], eps_l2_sb[:CL])
            den = wsp.tile([CL, Bi], F32, tag="den")
            nc.scalar.activation(den[:], s1s2_sb[0:CL, Bi:2 * Bi], AF.Ln)
            nc.scalar.activation(den[:], den[:], AF.Exp, scale=0.5)
            nc.scalar.add(den[:], den[:], eps_rl2_sb[:CL])
            nc.vector.tensor_scalar_mul(den[:], den[:], sq3[:])
            rden = wsp.tile([CL, Bi], F32, tag="rden")
            nc.vector.reciprocal_approx_fast(rden[:], den[:])
            nc.vector.tensor_mul(sims_sb[:], rden[:], s1_sb[:])

            nc.sync.dma_start(out_ext[:, :], sims_sb[:])

    nc.compile()
    return nc


def _prep_inputs(img_embed, cap_embed, Wg1, bg1, Wg2, bg2, Wb1, bb1, Wb2, bb2,
                 lens):
    """Host-side layout prep + per-core sharding. Returns in_maps (list of 8)."""
    f32 = np.float32
    imgT = np.ascontiguousarray(
        np.transpose(np.asarray(img_embed, f32), (2, 0, 1))).reshape(D, BR)
    capf = np.asarray(cap_embed, f32)
    lensf = np.asarray(lens)
    # ragged-mean weights: wfull[c, t] = (t < lens[c]) / lens[c]
    wfull = ((np.arange(T)[None, :] < lensf[:, None]) /
             lensf[:, None].astype(f32)).astype(f32)

    # one-hot static part of the masked-column lhsT tiles
    lhs8s = np.zeros((P, ND, CL, 8), f32)
    for c in range(CL):
        lhs8s[:, :, c, c] = 1.0
    shared = {
        "imgT": imgT,
        "Wg1": np.ascontiguousarray(np.asarray(Wg1, f32)),
        "Wg2": np.ascontiguousarray(np.asarray(Wg2, f32)),
        "Wb1": np.ascontiguousarray(np.asarray(Wb1, f32)),
        # beta path pre-scaled by GAMMA: the kernel computes iv' = GAMMA*iv,
        # to which sims is invariant (uniform scale cancels in s1/sqrt(s2))
        "Wb2": np.ascontiguousarray(np.asarray(Wb2, f32) * GAMMA),
        "bias_pack": np.ascontiguousarray(np.concatenate([
            np.asarray(bg1, f32).reshape(H, 1),
            np.asarray(bb1, f32).reshape(H, 1),
            np.asarray(bg2, f32).reshape(ND, P).T,
            np.asarray(bb2, f32).reshape(ND, P).T * GAMMA,
        ], axis=1)),
        "lhs8s": np.ascontiguousarray(lhs8s.reshape(P, ND * CL * 8)),
    }
    in_maps = []
    for i in range(NCORES):
        cs = slice(i * CL, (i + 1) * CL)
        cap_local = np.ascontiguousarray(capf[cs].reshape(CL * T, D))
        # block-diagonal mask-weight matrix [(c,t), c']
        wmat = np.zeros((CL * T, CL), f32)
        for cl in range(CL):
            wmat[cl * T:(cl + 1) * T, cl] = wfull[i * CL + cl]
        in_maps.append({**shared, "cap": cap_local, "wm": wmat})
    return in_maps


def kernel(**inputs) -> np.ndarray:
    global _COMPILED
    from concourse.bass_utils import run_bass_kernel_spmd

    if _COMPILED is None:
        _COMPILED = _build_graph()
    nc = _COMPILED

    in_maps = _prep_inputs(**inputs)
    res = run_bass_kernel_spmd(nc, in_maps, core_ids=list(range(NCORES)))
    sims = np.empty((Bi, Bc), np.float32)
    for i in range(NCORES):
        sims[:, i * CL:(i + 1) * CL] = res.results[i]["out"].T
    return sims


if __name__ == "__main__":
    # smoke test with random data
    rng = np.random.default_rng(0)
    ins = {
        "img_embed": rng.standard_normal((Bi, R, D), np.float32),
        "cap_embed": rng.standard_normal((Bc, T, D), np.float32),
        "Wg1": rng.standard_normal((D, H), np.float32) * 0.02,
        "bg1": np.zeros(H, np.float32),
        "Wg2": rng.standard_normal((H, D), np.float32) * 0.02,
        "bg2": np.zeros(D, np.float32),
        "Wb1": rng.standard_normal((D, H), np.float32) * 0.02,
        "bb1": np.zeros(H, np.float32),
        "Wb2": rng.standard_normal((H, D), np.float32) * 0.02,
        "bb2": np.zeros(D, np.float32),
        "lens": rng.integers(4, T - 4, Bc).astype(np.int32),
    }
    out = kernel(**ins)
    print(out.shape, out.dtype, np.abs(out).mean())
